# revision 2
# baseline (speedup 1.0000x reference)
"""BEVPoolV2 (segment_reduce) on 8 Trainium2 NeuronCores.

Contract: kernel(**inputs) takes FULL unsharded inputs (depth, feat,
ranks_depth, ranks_feat, maxn) and returns the FULL (1,1,200,200,64) f32
output.

Strategy (per the voxel-slab sharding):
  - 40000 output voxels x 40 points; core k owns voxels [5000k, 5000(k+1)),
    padded to 5120 = 40 blocks x 128 voxels. No cross-core accumulation.
  - Per block (128 voxels = 5120 points), SPMD on all 8 cores:
      * dma_gather feat rows (256B each) by ranks_feat -> [128, 40, 64]
        (voxel-per-partition, point-slot along free dim; the index stream is
        host-permuted so gathered row i lands at [i%128, i//128]).
      * dma_gather 256B depth rows at ranks_depth//64 -> [128, 40, 64]; the
        exact scalar is selected on the Vector engine with a one-hot
        (iota == ranks_depth%64) mask + X-axis reduce.
      * product feat * depth, then segmented sum over the 40 slots via a
        strided tensor_reduce -> [128 voxels, 64]; DMA to HBM.
  - Raw bass (manual rotating semaphores, double buffering).
"""
import sys
sys.path.insert(0, '/opt/trn_rl_repo')
import numpy as np
import concourse.bass as bass
import concourse.bacc as bacc
from concourse import mybir
from contextlib import ExitStack

P = 128
C = 64
MAXN = 40
V_TOT = 40000            # 1*200*200 output voxels
N_CORES = 8
NV_CORE = V_TOT // N_CORES   # 5000
NB = 40                      # blocks per core (5120 padded voxels)
NVP = NB * P
PTS_BLK = P * MAXN           # 5120 points per block
DEPTH_LEN = 498432           # 1*6*118*16*44
DEP_ROWS = 7792              # ceil((DEPTH_LEN+1)/64) padded
FEAT_ROWS = 4225             # 4224 + zero row
FEAT_PAD_IDX = 4224
DEP_PAD_IDX = DEPTH_LEN
IDXW = PTS_BLK // 16         # 320
GIDX = 512                   # idxs per dma_gather call
f32 = mybir.dt.float32
i16 = mybir.dt.int16


def _wrap_idx(idx, dtype=np.int16):
    """[..., N] -> [..., 128, N//16]: idx i at [i%16, i//16], replicated x8."""
    n = idx.shape[-1]
    w = idx.reshape(*idx.shape[:-1], n // 16, 16)
    w = np.swapaxes(w, -1, -2)
    w = np.broadcast_to(w[..., None, :, :], (*idx.shape[:-1], 8, 16, n // 16))
    return np.ascontiguousarray(
        w.reshape(*idx.shape[:-1], 128, n // 16)).astype(dtype)


def _host_prep(depth, feat, ranks_depth, ranks_feat):
    depth_flat = np.asarray(depth, np.float32).reshape(-1)
    dep_tab = np.zeros((DEP_ROWS, C), np.float32)
    dep_tab.reshape(-1)[:DEPTH_LEN] = depth_flat
    feat_tab = np.zeros((FEAT_ROWS, C), np.float32)
    feat_tab[:FEAT_ROWS - 1] = np.asarray(feat, np.float32).reshape(-1, C)
    iota = np.broadcast_to(np.arange(C, dtype=np.float32), (P, C)).copy()

    rd = np.asarray(ranks_depth, np.int64).reshape(V_TOT, MAXN)
    rf = np.asarray(ranks_feat, np.int64).reshape(V_TOT, MAXN)

    in_maps = []
    for k in range(N_CORES):
        rd_c = np.full((NVP, MAXN), DEP_PAD_IDX, np.int64)
        rf_c = np.full((NVP, MAXN), FEAT_PAD_IDX, np.int64)
        rd_c[:NV_CORE] = rd[k * NV_CORE:(k + 1) * NV_CORE]
        rf_c[:NV_CORE] = rf[k * NV_CORE:(k + 1) * NV_CORE]
        rd_b = rd_c.reshape(NB, P, MAXN).transpose(0, 2, 1)   # [NB, s, p]
        rf_b = rf_c.reshape(NB, P, MAXN).transpose(0, 2, 1)
        in_maps.append({
            "dep_tab": dep_tab,
            "feat_tab": feat_tab,
            "rf_w": _wrap_idx(rf_b.reshape(NB, PTS_BLK)),
            "rdhi_w": _wrap_idx((rd_b // C).reshape(NB, PTS_BLK)),
            "rdlo": np.ascontiguousarray(
                (rd_b % C).astype(np.float32).transpose(0, 2, 1)),
            "iota": iota,
        })
    return in_maps


def _build_kernel(nb=NB, m_rep=1, gidx=GIDX, scratch=16384):
    gch = PTS_BLK // gidx        # gather chunks per table per block
    ginc = 16 * 2 * gch          # s_gat increments per block
    sl, wl = gidx // 128, gidx // 16
    nt = nb * m_rep

    nc = bacc.Bacc("TRN2", debug=False, dynamic_dma_scratch_size=scratch)
    dep_tab = nc.dram_tensor("dep_tab", [DEP_ROWS, C], f32, kind="ExternalInput")
    feat_tab = nc.dram_tensor("feat_tab", [FEAT_ROWS, C], f32, kind="ExternalInput")
    rf_w = nc.dram_tensor("rf_w", [nb, P, IDXW], i16, kind="ExternalInput")
    rdhi_w = nc.dram_tensor("rdhi_w", [nb, P, IDXW], i16, kind="ExternalInput")
    rdlo = nc.dram_tensor("rdlo", [nb, P, MAXN], f32, kind="ExternalInput")
    iota = nc.dram_tensor("iota", [P, C], f32, kind="ExternalInput")
    out = nc.dram_tensor("out", [nb, P, C], f32, kind="ExternalOutput")

    with ExitStack() as st:
        e = st.enter_context
        rf_sb = e(nc.sbuf_tensor("rf_sb", [P, 2, IDXW], i16))
        rdhi_sb = e(nc.sbuf_tensor("rdhi_sb", [P, 2, IDXW], i16))
        rdlo_sb = e(nc.sbuf_tensor("rdlo_sb", [P, 2, MAXN], f32))
        gf_sb = e(nc.sbuf_tensor("gf_sb", [P, 2, MAXN, C], f32))
        gd_sb = e(nc.sbuf_tensor("gd_sb", [P, 2, MAXN, C], f32))
        mask_sb = e(nc.sbuf_tensor("mask_sb", [P, MAXN, C], f32))
        d_sb = e(nc.sbuf_tensor("d_sb", [P, MAXN], f32))
        o_sb = e(nc.sbuf_tensor("o_sb", [P, 2, C], f32))
        iota_sb = e(nc.sbuf_tensor("iota_sb", [P, C], f32))
        s_cst = e(nc.semaphore("s_cst"))
        s_idx = [e(nc.semaphore(f"s_idx{j}")) for j in range(2)]
        s_gat = [e(nc.semaphore(f"s_gat{j}")) for j in range(2)]
        s_out = [e(nc.semaphore(f"s_out{j}")) for j in range(2)]
        s_dve = e(nc.semaphore("s_dve"))
        blk = e(nc.Block())

        @blk.sync
        def _(sync):
            sync.dma_start(iota_sb[:], iota[:]).then_inc(s_cst, 16)
            for t in range(nt):
                b, j, k = t % nb, t % 2, t // 2
                if t >= 2:
                    sync.wait_ge(s_idx[j], 48 * k)
                    sync.wait_ge(s_gat[j], ginc * k)
                    sync.wait_ge(s_dve, t - 1)
                sync.dma_start(rf_sb[:, j], rf_w[b]).then_inc(s_idx[j], 16)
                sync.dma_start(rdhi_sb[:, j], rdhi_w[b]).then_inc(s_idx[j], 16)
                sync.dma_start(rdlo_sb[:, j], rdlo[b]).then_inc(s_idx[j], 16)
                if t >= 1:
                    tt = t - 1
                    jj, kk = tt % 2, tt // 2
                    sync.wait_ge(s_dve, tt + 1)
                    if tt >= 2:
                        sync.wait_ge(s_out[jj], 16 * kk)
                    sync.dma_start(out[tt % nb], o_sb[:, jj]).then_inc(s_out[jj], 16)
            tt = nt - 1
            jj, kk = tt % 2, tt // 2
            sync.wait_ge(s_dve, nt)
            if tt >= 2:
                sync.wait_ge(s_out[jj], 16 * kk)
            sync.dma_start(out[tt % nb], o_sb[:, jj]).then_inc(s_out[jj], 16)

        @blk.gpsimd
        def _(gpsimd):
            for t in range(nt):
                j, k = t % 2, t // 2
                gpsimd.wait_ge(s_idx[j], 48 * (k + 1))
                if t >= 2:
                    gpsimd.wait_ge(s_gat[j], ginc * k)
                    gpsimd.wait_ge(s_dve, t - 1)
                for c in range(gch):
                    gpsimd.dma_gather(
                        gf_sb[:, j, sl*c:sl*(c+1)], feat_tab[:],
                        rf_sb[:, j, wl*c:wl*(c+1)], gidx, gidx, C
                    ).then_inc(s_gat[j], 16)
                    gpsimd.dma_gather(
                        gd_sb[:, j, sl*c:sl*(c+1)], dep_tab[:],
                        rdhi_sb[:, j, wl*c:wl*(c+1)], gidx, gidx, C
                    ).then_inc(s_gat[j], 16)

        @blk.vector
        def _(vector):
            for t in range(nt):
                j, k = t % 2, t // 2
                if t == 0:
                    vector.wait_ge(s_cst, 16)
                vector.wait_ge(s_idx[j], 48 * (k + 1))
                vector.tensor_tensor(
                    out=mask_sb[:],
                    in0=iota_sb[:][:, None, :].to_broadcast([P, MAXN, C]),
                    in1=rdlo_sb[:, j][:, :, None].to_broadcast([P, MAXN, C]),
                    op=mybir.AluOpType.is_equal)
                vector.wait_ge(s_gat[j], ginc * (k + 1))
                vector.drain()
                vector.tensor_tensor(out=mask_sb[:], in0=mask_sb[:],
                                     in1=gd_sb[:, j], op=mybir.AluOpType.mult)
                vector.drain()
                vector.reduce_sum(out=d_sb[:], in_=mask_sb[:],
                                  axis=mybir.AxisListType.X)
                vector.drain()
                vector.tensor_tensor(
                    out=gf_sb[:, j], in0=gf_sb[:, j],
                    in1=d_sb[:][:, :, None].to_broadcast([P, MAXN, C]),
                    op=mybir.AluOpType.mult)
                vector.drain()
                if t >= 2:
                    vector.wait_ge(s_out[j], 16 * k)
                vector.reduce_sum(
                    out=o_sb[:, j],
                    in_=gf_sb[:, j].rearrange("p s c -> p c s"),
                    axis=mybir.AxisListType.X).then_inc(s_dve, 1)

    nc.compile()
    return nc


_NC_CACHE = None


def kernel(depth, feat, ranks_depth, ranks_feat, maxn):
    global _NC_CACHE
    from concourse.bass_utils import run_bass_kernel_spmd
    assert int(maxn) == MAXN
    in_maps = _host_prep(depth, feat, ranks_depth, ranks_feat)
    if _NC_CACHE is None:
        _NC_CACHE = _build_kernel()
    res = run_bass_kernel_spmd(_NC_CACHE, in_maps, core_ids=list(range(N_CORES)))
    parts = [r["out"].reshape(NVP, C)[:NV_CORE] for r in res.results]
    return np.concatenate(parts, 0).reshape(1, 1, 200, 200, C)


# revision 6
# speedup vs baseline: 1.0400x; 1.0400x over previous
"""BEVPoolV2 (segment_reduce) on 8 Trainium2 NeuronCores.

Contract: kernel(**inputs) takes FULL unsharded inputs (depth, feat,
ranks_depth, ranks_feat, maxn) and returns the FULL (1,1,200,200,64) f32
output.

Strategy (per the voxel-slab sharding):
  - 40000 output voxels x 40 points; core k owns voxels [5000k, 5000(k+1)),
    padded to 5120 = 40 blocks x 128 voxels. No cross-core accumulation.
  - Per block (128 voxels = 5120 points), SPMD on all 8 cores:
      * dma_gather feat rows (256B each) by ranks_feat -> [128, 40, 64]
        (voxel-per-partition, point-slot along free dim; the index stream is
        host-permuted so gathered row i lands at [i%128, i//128]).
      * dma_gather 256B depth rows at ranks_depth//64 -> [128, 40, 64]; the
        exact scalar is selected on the Vector engine with a one-hot
        (iota == ranks_depth%64) mask + X-axis reduce.
      * product feat * depth, then segmented sum over the 40 slots via a
        strided tensor_reduce -> [128 voxels, 64]; DMA to HBM.
  - SWDGE descriptor generation is the bottleneck (~9ns/idx/queue), so the
    gathers are spread over 4 SWDGE queues (near-linear speedup): depth on
    queues {2,3}, feat on {0,1}, rotated per block for balance. Separate
    completion semaphores per stream let the Vector depth-select start as
    soon as the depth gathers land, before the feat stream finishes.
  - Raw bass (manual rotating semaphores, double buffering).
"""
import sys
sys.path.insert(0, '/opt/trn_rl_repo')
import numpy as np
import concourse.bass as bass
import concourse.bacc as bacc
from concourse import mybir
from contextlib import ExitStack

P = 128
C = 64
MAXN = 40
V_TOT = 40000            # 1*200*200 output voxels
N_CORES = 8
NV_CORE = V_TOT // N_CORES   # 5000
NB = 40                      # blocks per core (5120 padded voxels)
NVP = NB * P
PTS_BLK = P * MAXN           # 5120 points per block
DEPTH_LEN = 498432           # 1*6*118*16*44
DEP_ROWS = 7792              # ceil((DEPTH_LEN+1)/64) padded
FEAT_ROWS = 4225             # 4224 + zero row
FEAT_PAD_IDX = 4224
DEP_PAD_IDX = DEPTH_LEN
IDXW = PTS_BLK // 16         # 320
GIDX = 1024                  # idxs per dma_gather call
GCH = PTS_BLK // GIDX        # 5 gather calls per table per block
f32 = mybir.dt.float32
i16 = mybir.dt.int16


def _wrap_idx(idx, dtype=np.int16):
    """[..., N] -> [..., 128, N//16]: idx i at [i%16, i//16], replicated x8."""
    n = idx.shape[-1]
    w = idx.reshape(*idx.shape[:-1], n // 16, 16)
    w = np.swapaxes(w, -1, -2)
    w = np.broadcast_to(w[..., None, :, :], (*idx.shape[:-1], 8, 16, n // 16))
    return np.ascontiguousarray(
        w.reshape(*idx.shape[:-1], 128, n // 16)).astype(dtype)


def _host_prep(depth, feat, ranks_depth, ranks_feat):
    depth_flat = np.asarray(depth, np.float32).reshape(-1)
    dep_tab = np.zeros((DEP_ROWS, C), np.float32)
    dep_tab.reshape(-1)[:DEPTH_LEN] = depth_flat
    feat_tab = np.zeros((FEAT_ROWS, C), np.float32)
    feat_tab[:FEAT_ROWS - 1] = np.asarray(feat, np.float32).reshape(-1, C)
    iota = np.broadcast_to(np.arange(C, dtype=np.float32), (P, C)).copy()

    rd = np.asarray(ranks_depth, np.int64).reshape(V_TOT, MAXN)
    rf = np.asarray(ranks_feat, np.int64).reshape(V_TOT, MAXN)

    in_maps = []
    for k in range(N_CORES):
        rd_c = np.full((NVP, MAXN), DEP_PAD_IDX, np.int64)
        rf_c = np.full((NVP, MAXN), FEAT_PAD_IDX, np.int64)
        rd_c[:NV_CORE] = rd[k * NV_CORE:(k + 1) * NV_CORE]
        rf_c[:NV_CORE] = rf[k * NV_CORE:(k + 1) * NV_CORE]
        rd_b = rd_c.reshape(NB, P, MAXN).transpose(0, 2, 1)   # [NB, s, p]
        rf_b = rf_c.reshape(NB, P, MAXN).transpose(0, 2, 1)
        in_maps.append({
            "dep_tab": dep_tab,
            "feat_tab": feat_tab,
            "rf_w": _wrap_idx(rf_b.reshape(NB, PTS_BLK)),
            "rdhi_w": _wrap_idx((rd_b // C).reshape(NB, PTS_BLK)),
            "rdlo": np.ascontiguousarray(
                (rd_b % C).astype(np.float32).transpose(0, 2, 1)),
            "iota": iota,
        })
    return in_maps


def _build_kernel(nb=NB, m_rep=1, gidx=GIDX, scratch=32768, queues=4):
    gch = PTS_BLK // gidx        # gather chunks per table per block
    ginc = 16 * gch              # per-stream s_gat increments per block
    sl, wl = gidx // 128, gidx // 16
    nt = nb * m_rep

    nc = bacc.Bacc("TRN2", debug=False, dynamic_dma_scratch_size=scratch,
                   num_swdge_queues=queues)
    dep_tab = nc.dram_tensor("dep_tab", [DEP_ROWS, C], f32, kind="ExternalInput")
    feat_tab = nc.dram_tensor("feat_tab", [FEAT_ROWS, C], f32, kind="ExternalInput")
    rf_w = nc.dram_tensor("rf_w", [nb, P, IDXW], i16, kind="ExternalInput")
    rdhi_w = nc.dram_tensor("rdhi_w", [nb, P, IDXW], i16, kind="ExternalInput")
    rdlo = nc.dram_tensor("rdlo", [nb, P, MAXN], f32, kind="ExternalInput")
    iota = nc.dram_tensor("iota", [P, C], f32, kind="ExternalInput")
    out = nc.dram_tensor("out", [nb, P, C], f32, kind="ExternalOutput")

    with ExitStack() as st:
        e = st.enter_context
        rf_sb = e(nc.sbuf_tensor("rf_sb", [P, 2, IDXW], i16))
        rdhi_sb = e(nc.sbuf_tensor("rdhi_sb", [P, 2, IDXW], i16))
        rdlo_sb = e(nc.sbuf_tensor("rdlo_sb", [P, 2, MAXN], f32))
        gf_sb = e(nc.sbuf_tensor("gf_sb", [P, 2, MAXN, C], f32))
        gd_sb = e(nc.sbuf_tensor("gd_sb", [P, 2, MAXN, C], f32))
        mask_sb = e(nc.sbuf_tensor("mask_sb", [P, MAXN, C], f32))
        d_sb = e(nc.sbuf_tensor("d_sb", [P, MAXN], f32))
        o_sb = e(nc.sbuf_tensor("o_sb", [P, 2, C], f32))
        iota_sb = e(nc.sbuf_tensor("iota_sb", [P, C], f32))
        s_cst = e(nc.semaphore("s_cst"))
        s_idx = [e(nc.semaphore(f"s_idx{j}")) for j in range(2)]
        s_gf = [e(nc.semaphore(f"s_gf{j}")) for j in range(2)]
        s_gd = [e(nc.semaphore(f"s_gd{j}")) for j in range(2)]
        s_out = [e(nc.semaphore(f"s_out{j}")) for j in range(2)]
        s_dve = e(nc.semaphore("s_dve"))
        blk = e(nc.Block())

        @blk.sync
        def _(sync):
            sync.dma_start(iota_sb[:], iota[:]).then_inc(s_cst, 16)
            for t in range(nt):
                b, j, k = t % nb, t % 2, t // 2
                if t >= 2:
                    sync.wait_ge(s_idx[j], 48 * k)
                    sync.wait_ge(s_gf[j], ginc * k)
                    sync.wait_ge(s_gd[j], ginc * k)
                    sync.wait_ge(s_dve, t - 1)
                sync.dma_start(rdlo_sb[:, j], rdlo[b]).then_inc(s_idx[j], 16)
                sync.dma_start(rdhi_sb[:, j], rdhi_w[b]).then_inc(s_idx[j], 16)
                sync.dma_start(rf_sb[:, j], rf_w[b]).then_inc(s_idx[j], 16)
                if t >= 1:
                    tt = t - 1
                    jj, kk = tt % 2, tt // 2
                    sync.wait_ge(s_dve, tt + 1)
                    if tt >= 2:
                        sync.wait_ge(s_out[jj], 16 * kk)
                    sync.dma_start(out[tt % nb], o_sb[:, jj]).then_inc(s_out[jj], 16)
            tt = nt - 1
            jj, kk = tt % 2, tt // 2
            sync.wait_ge(s_dve, nt)
            if tt >= 2:
                sync.wait_ge(s_out[jj], 16 * kk)
            sync.dma_start(out[tt % nb], o_sb[:, jj]).then_inc(s_out[jj], 16)

        @blk.gpsimd
        def _(gpsimd):
            for t in range(nt):
                j, k = t % 2, t // 2
                gpsimd.wait_ge(s_idx[j], 48 * (k + 1))
                if t >= 2:
                    gpsimd.wait_ge(s_gf[j], ginc * k)
                    gpsimd.wait_ge(s_gd[j], ginc * k)
                    gpsimd.wait_ge(s_dve, t - 1)
                # depth first (vector consumes gd before gf); rotate queues
                # per block so all 4 stay balanced with 5 calls per stream.
                for c in range(gch):
                    gpsimd.dma_gather(
                        gd_sb[:, j, sl*c:sl*(c+1)], dep_tab[:],
                        rdhi_sb[:, j, wl*c:wl*(c+1)], gidx, gidx, C,
                        queue_num=2 + (t * gch + c) % 2,
                    ).then_inc(s_gd[j], 16)
                for c in range(gch):
                    gpsimd.dma_gather(
                        gf_sb[:, j, sl*c:sl*(c+1)], feat_tab[:],
                        rf_sb[:, j, wl*c:wl*(c+1)], gidx, gidx, C,
                        queue_num=(t * gch + c) % 2,
                    ).then_inc(s_gf[j], 16)

        @blk.vector
        def _(vector):
            for t in range(nt):
                j, k = t % 2, t // 2
                if t == 0:
                    vector.wait_ge(s_cst, 16)
                vector.wait_ge(s_idx[j], 48 * k + 16)  # rdlo is the 1st idx DMA
                vector.tensor_tensor(
                    out=mask_sb[:],
                    in0=iota_sb[:][:, None, :].to_broadcast([P, MAXN, C]),
                    in1=rdlo_sb[:, j][:, :, None].to_broadcast([P, MAXN, C]),
                    op=mybir.AluOpType.is_equal)
                vector.wait_ge(s_gd[j], ginc * (k + 1))
                vector.drain()
                vector.tensor_tensor(out=mask_sb[:], in0=mask_sb[:],
                                     in1=gd_sb[:, j], op=mybir.AluOpType.mult)
                vector.drain()
                vector.reduce_sum(out=d_sb[:], in_=mask_sb[:],
                                  axis=mybir.AxisListType.X)
                vector.drain()
                vector.wait_ge(s_gf[j], ginc * (k + 1))
                vector.tensor_tensor(
                    out=gf_sb[:, j], in0=gf_sb[:, j],
                    in1=d_sb[:][:, :, None].to_broadcast([P, MAXN, C]),
                    op=mybir.AluOpType.mult)
                vector.drain()
                if t >= 2:
                    vector.wait_ge(s_out[j], 16 * k)
                vector.reduce_sum(
                    out=o_sb[:, j],
                    in_=gf_sb[:, j].rearrange("p s c -> p c s"),
                    axis=mybir.AxisListType.X).then_inc(s_dve, 1)

    nc.compile()
    return nc


_NC_CACHE = None


def kernel(depth, feat, ranks_depth, ranks_feat, maxn):
    global _NC_CACHE
    from concourse.bass_utils import run_bass_kernel_spmd
    assert int(maxn) == MAXN
    in_maps = _host_prep(depth, feat, ranks_depth, ranks_feat)
    if _NC_CACHE is None:
        _NC_CACHE = _build_kernel()
    res = run_bass_kernel_spmd(_NC_CACHE, in_maps, core_ids=list(range(N_CORES)))
    parts = [r["out"].reshape(NVP, C)[:NV_CORE] for r in res.results]
    return np.concatenate(parts, 0).reshape(1, 1, 200, 200, C)
